# revision 46
# baseline (speedup 1.0000x reference)
"""Trainium2 Bass kernel for nn_CapsuleLayer (B=32, In=128, Din=256, ch=32, Nc=47, Dc=64).

Sharding: over the OUTPUT-CAPSULE axis Nc (47 -> pad 48 = 8 cores x 6 capsules).
Routing is fully independent per (batch, output-capsule), and W (94 MiB) is the
dominant HBM tensor -- Nc-sharding reads W exactly once total (12.6 MiB/core)
instead of replicating it 8x as batch-sharding would.

bf16 everywhere the 2e-2 tolerance allows (stream, inputs_hat, routing
elementwise); fp32 for PSUM accumulation, softmax normalization and squash.
Measured end-to-end numeric error of this pipeline ~5e-3.

Per-core layout: inputs_hat IH[p=(b,rr) 128 partitions, (c, k, n)] bf16 --
c-major so that
  * the a-step mul  TMP = IH * OUTr   broadcasts OUTr over the OUTER c axis
    (innermost reads stay step-1 -> DVE 2x bf16 mode),
  * the s-step mul  TS = IH * E       broadcasts E[p,c,n] over the MIDDLE k
    axis (innermost n runs step-1 -> 2x mode),
  * per-channel [128, (k,n)] blocks stay contiguous for PSUM copies and
    matmul rhs operands.
The k-reduction of the a-step runs as a pairwise TREE of 2x tensor_adds
(tensor_reduce is capped at 1x mode); the c+rr reductions of the s-step run
on the PE as PSUM-accumulated block-diagonal (BD4) matmuls.  sqrt in squash
is computed as exp(-0.5*ln) so ACT needs only the natural_log_exp table set
(no per-iteration table reloads).

Toolchain constraint: matmul (S3_LW) and DMA (DIRECT2D) instructions accept at
most ONE sync wait at codegen; DVE likewise.  Deps from the SAME engine merge
into one sem, so the kernel keeps every matmul's waits on a single engine;
const-DMA sems and psum-slot WAR ticks are pre-absorbed into the PE clock via
tiny dummy matmuls, and cross-engine (ACT<->DVE) handoffs via tiny copies.
"""

import numpy as np

B, IN, DIN = 32, 128, 256
CH, NC, DC = 32, 47, 64
NCP = 48          # padded Nc
NSH = 6           # capsules per core
NCORES = 8
NK = NSH * DC     # 384
EPS = 1e-7

_cache = {}


def _build_nc():
    import concourse.bass as bass
    import concourse.tile as tile
    from concourse import mybir
    from concourse.tile_rust import add_dep_helper

    f32 = mybir.dt.float32
    bf16 = mybir.dt.bfloat16
    nc = bass.Bass()

    # packed stream, PARTITION-MAJOR: xw[d, cd, (xt 128 | wt 384)] so each
    # DMA group reads one contiguous block per partition (few descriptors)
    xw = nc.dram_tensor("xw", [128, CH * 2, 512], bf16, kind="ExternalInput")
    # fp32 consts [bd4 | bd4t(rows<32) | brep(rows<32)]
    cstf = nc.dram_tensor("cstf", [128, 544], f32, kind="ExternalInput")
    out_d = nc.dram_tensor("out", [B, NK], f32, kind="ExternalOutput")

    ADD = mybir.AluOpType.add
    MULT = mybir.AluOpType.mult
    AX = mybir.AxisListType.X
    AF = mybir.ActivationFunctionType

    with tile.TileContext(nc) as tc:
        with (
            tc.tile_pool(name="singles", bufs=1) as singles,
            tc.tile_pool(name="work", bufs=1) as work,
            tc.tile_pool(name="small", bufs=2) as small,
            tc.tile_pool(name="ps_ih", bufs=4, space="PSUM") as ps_ih,
            tc.tile_pool(name="ps_s1", bufs=1, space="PSUM") as ps_s1,
            tc.tile_pool(name="ps_sm", bufs=1, space="PSUM") as ps_sm,
            tc.tile_pool(name="ps_s2", bufs=2, space="PSUM") as ps_s2,
        ):
            cstf_t = singles.tile([128, 544], f32)
            bd4f_t = cstf_t[:, 0:B]             # fp32 [128, 32]
            bd4tf_t = cstf_t[0:B, B:B + 128]    # fp32 [32, 128]
            brep_t = cstf_t[0:B, B + 128:B + 128 + NK]  # fp32 [32, 384] (k,n)
            eps_t = singles.tile([B, 1], f32)
            nc.vector.memset(eps_t[:], EPS)

            # IH[p, c, k, n] bf16 -- c-major inputs_hat
            IH = singles.tile([128, CH, DC, NSH], bf16)
            STREAM = singles.tile([128, CH * 2, 512], bf16)

            # ---------------- phase 1: inputs_hat + iter-1 s ----------------
            # cstf first (tiny, unblocks PE warm-up; rides the ACT engine's
            # HWDGE queues), then 6 stream DMA groups with a window-3 chain
            # (g waits g-3): a single dma_start only reaches ~200 GB/s, a few
            # in flight saturate HBM, and chaining staggers the completions
            # so the PE can trail each group.
            cf_dma = nc.scalar.dma_start(out=cstf_t[:], in_=cstf[:])
            # 6 groups keeps total HWDGE DMA count at 8 (= semaphore pool
            # size) so the output DMA gets a fresh completion semaphore.
            GSZ = (4, 4, 6, 6, 6, 6)        # channels per DMA group
            GST = []                        # group start channel
            s_dmas = []
            cs0 = 0
            for g, gsz in enumerate(GSZ):
                GST.append(cs0)
                if g >= 3:
                    # chain via a sync-queue nop: the serial sync queue then
                    # holds this group's issue until g-3 completes, keeping
                    # the DMA instruction's single wait slot free.
                    chn = nc.sync.nop()
                    add_dep_helper(chn.ins, s_dmas[g - 3].ins, sync=True,
                                   reason="window-3 stream DMA chain")
                sd = nc.sync.dma_start(
                    out=STREAM[:, 2 * cs0:2 * (cs0 + gsz), :],
                    in_=xw[:, 2 * cs0:2 * (cs0 + gsz), :],
                )
                s_dmas.append(sd)
                cs0 += gsz
            # bf16 bd4 derived on-device; this DVE cast also pre-observes the
            # cstf-DMA sem for every later DVE reader of cstf (1-wait limit).
            bd4b_t = singles.tile([128, 32], bf16)
            nc.vector.tensor_copy(bd4b_t[:], cstf_t[:, 0:B])

            # Absorb the const-DMA sem into the PE clock (dummy matmul) so
            # real matmuls carry a single wait; then ~10 junk matmuls keep PE
            # busy through the HAM warm-up window while the stream DMA lands.
            # pd packs the dummy target (cols 0:2) and pz (cols 2:8) into one
            # PSUM slot to stay within the 8-bank budget.
            pd = ps_sm.tile([B, 8], f32, tag="dummy")
            last_dummy = nc.tensor.matmul(
                pd[:2, 0:2], cstf_t[:2, :2], cstf_t[:2, :2], start=True, stop=True,
                skip_group_check=True,
            )
            for _ in range(10):
                last_dummy = nc.tensor.matmul(
                    pd[:2, 0:2], cstf_t[:2, :2], cstf_t[:2, :2], start=True,
                    stop=True, skip_group_check=True,
                )

            psum_s1 = ps_s1.tile([B, NK], f32)

            # All PSUM->IH copies ride DVE: each channel's s1-matmul then
            # waits on a newer DVE sem value than any later start-matmul's
            # psum WAR needs, so those WARs are elided and every ih matmul
            # keeps a single wait (its PSUM-WAW self-tick).  Group-first
            # matmuls additionally need the stream-DMA sem: absorb it into
            # the PE clock with a dummy matmul.
            copy_insts = []
            for c in range(CH):
                if c > 0 and c in GST:
                    dmy = nc.tensor.matmul(pd[:2, 0:2], cstf_t[:2, :2], cstf_t[:2, :2],
                                           start=True, stop=True,
                                           skip_group_check=True)
                    add_dep_helper(dmy.ins, s_dmas[GST.index(c)].ins,
                                   sync=True,
                                   reason="absorb stream DMA sem on PE")
                    last_dummy = dmy
                psum_ih = ps_ih.tile([128, NK], f32, tag="ih")
                for dc in range(2):
                    cd = c * 2 + dc
                    mih = nc.tensor.matmul(
                        psum_ih[:], STREAM[:, cd, 0:128], STREAM[:, cd, 128:512],
                        start=(dc == 0), stop=(dc == 1),
                    )
                    if dc == 0:
                        add_dep_helper(mih.ins, last_dummy.ins, sync=False,
                                       reason="order dummy before matmul")
                pv = psum_ih[:].rearrange("p (k n) -> p k n", n=NSH)
                # ACT does all PSUM->IH copies: it is otherwise idle in phase
                # 1, sits closer to PSUM, and keeps the DVE free; the lagged
                # s1-matmul keeps every start-matmul's psum WAR elided (same
                # monotonic ACT sem).
                copy_insts.append(nc.scalar.copy(IH[:, c], pv))
                # iter-1 s accumulation (sum_rr on PE via BD4, sum_c via PSUM
                # accumulation), LAGGED two channels so the matmul never
                # stalls on the freshly-issued DVE copy.
                if c >= 2:
                    nc.tensor.matmul(
                        psum_s1[:], bd4b_t[:],
                        IH[:, c - 2].rearrange("p k n -> p (k n)"),
                        start=(c == 2), stop=False,
                        skip_group_check=True,
                    )
            for c in (CH - 2, CH - 1):
                nc.tensor.matmul(
                    psum_s1[:], bd4b_t[:],
                    IH[:, c].rearrange("p k n -> p (k n)"),
                    start=False, stop=(c == CH - 1),
                    skip_group_check=True,
                )

            _absn = [0]

            def absorb(eng, src_ap):
                """Tiny copy on `eng` reading src_ap: pre-observes the
                producer's sem so the next real op keeps a single wait."""
                _absn[0] += 1
                scr = small.tile([2, 2], f32, tag="abs%d" % _absn[0])
                if eng == "v":
                    return nc.vector.tensor_copy(scr[:], src_ap)
                return nc.scalar.copy(scr[:], src_ap)

            def squash(S, tag, out_dtype=f32, out_tag="outf"):
                """S: [B, (k,n)] fp32 sbuf tile -> OUT [B, (k,n)]."""
                Ssq = work.tile([B, NK], f32, tag="Ssq")
                nc.vector.tensor_mul(Ssq[:], S[:], S[:])
                m2 = small.tile([B, NSH], f32, tag="m2")
                nc.vector.tensor_reduce(
                    m2[:], Ssq[:].rearrange("p (k n) -> p n k", n=NSH),
                    axis=AX, op=ADD,
                )
                d1 = small.tile([B, NSH], f32, tag="d1")
                nc.vector.tensor_scalar_add(d1[:], m2[:], 1.0)
                rd1 = small.tile([B, NSH], f32, tag="rd1")
                nc.vector.reciprocal(rd1[:], d1[:])
                absorb("s", m2[:2, :2])          # ACT clock <- m2 (DVE)
                # rsqrt(m2+eps) = exp(-0.5*ln(m2+eps)): keeps ACT on the
                # natural_log_exp table set (shared with softmax exp)
                lt = small.tile([B, NSH], f32, tag="lt")
                nc.scalar.activation(lt[:], m2[:], AF.Ln, bias=eps_t[:])
                rs = small.tile([B, NSH], f32, tag="rs")
                nc.scalar.activation(rs[:], lt[:], AF.Exp, scale=-0.5)
                absorb("v", rs[:2, :2])          # DVE clock <- rs (ACT)
                g0 = small.tile([B, NSH], f32, tag="g0")
                nc.vector.tensor_mul(g0[:], m2[:], rd1[:])
                g_ = small.tile([B, NSH], f32, tag="g")
                nc.vector.tensor_mul(g_[:], g0[:], rs[:])
                OUT = work.tile([B, NK], out_dtype, tag=out_tag)
                _sq_last[0] = nc.vector.tensor_mul(
                    OUT[:].rearrange("p (k n) -> p k n", n=NSH),
                    S[:].rearrange("p (k n) -> p k n", n=NSH),
                    g_[:].rearrange("p (o n) -> p o n", o=1)
                       .broadcast_to([B, DC, NSH]),
                )
                return OUT

            _sq_last = [None]

            # bf16 bd4t for the replicate matmuls (fp32 matmul is 4x slower)
            bd4tb_t = singles.tile([B, 128], bf16)
            nc.vector.tensor_copy(bd4tb_t[:], bd4tf_t)

            def replicate(OUTb, tag):
                """OUTb [B, NK] bf16 -> OUTr [128, NK] bf16 (row b -> 4b..).
                PSUM rides the (phase-1-dead) ps_ih slots; those carry a PSUM
                WAW self-tick, so absorb the OUT RAW into the PE clock first.
                """
                dmy = nc.tensor.matmul(pd[:2, 0:2], cstf_t[:2, :2], cstf_t[:2, :2],
                                       start=True, stop=True,
                                       skip_group_check=True)
                add_dep_helper(dmy.ins, _sq_last[0].ins, sync=True,
                               reason="absorb OUT RAW on PE")
                pr = ps_ih.tile([128, NK], f32, tag="ih")
                nc.tensor.matmul(pr[:], bd4tb_t[:], OUTb[:], start=True,
                                 stop=True, skip_group_check=True)
                R = work.tile([128, NK], bf16, tag="OUTr")
                nc.vector.tensor_copy(R[:], pr[:])
                return R

            # ---------------- iter 1 ----------------
            S1 = work.tile([B, NK], f32, tag="S")
            nc.vector.scalar_tensor_tensor(
                out=S1[:], in0=psum_s1[:], scalar=1.0 / IN, in1=brep_t,
                op0=MULT, op1=ADD,
            )
            OUT1 = squash(S1, "1", out_dtype=bf16, out_tag="outb")
            OUTr = replicate(OUT1, "1")

            A2 = None
            for it in (2, 3):
                # ---- a-step: TMP = IH * OUTr (2x bf16); tree-reduce over k
                if it == 2:
                    absorb("v", IH[:2, CH - 1, 0, 0:2])  # DVE clock <- ACT copies
                TMP = work.tile([128, CH, DC, NSH], bf16, tag="TMP")
                nc.vector.tensor_mul(
                    TMP[:].rearrange("p c k n -> p c (k n)"),
                    IH[:].rearrange("p c k n -> p c (k n)"),
                    OUTr[:].rearrange("p (o kn) -> p o kn", o=1)
                          .broadcast_to([128, CH, NK]),
                )
                T = TMP
                kk = DC
                while kk > 2:
                    kk //= 2
                    Tn = work.tile([128, CH, kk, NSH], bf16, tag="T%d" % kk)
                    nc.vector.tensor_add(Tn[:], T[:, :, 0:kk, :],
                                         T[:, :, kk:2 * kk, :])
                    T = Tn
                A = work.tile([128, CH, NSH], f32, tag="A%d" % it)
                nc.vector.tensor_add(A[:], T[:, :, 0, :], T[:, :, 1, :])
                if A2 is None:
                    BL = A
                    A2 = A
                else:
                    BL = work.tile([128, CH, NSH], f32, tag="BL")
                    nc.vector.tensor_add(BL[:], A[:], A2[:])
                # ---- E = exp(BL) (bf16 out)
                absorb("s", BL[:2, 0, 0:2])       # ACT clock <- BL (DVE)
                E = work.tile([128, CH, NSH], bf16, tag="E")
                nc.scalar.activation(E[:], BL[:], AF.Exp)
                # ---- Z: sum_c on DVE, sum_rr on PE
                absorb("v", E[:2, 0, 0:2])        # DVE clock <- E (ACT)
                Zp = small.tile([128, NSH], f32, tag="Zp")
                nc.vector.tensor_reduce(
                    Zp[:], E[:].rearrange("p c n -> p n c"), axis=AX, op=ADD,
                )
                pz = pd[:, 2:8]
                nc.tensor.matmul(pz, bd4f_t, Zp[:], start=True, stop=True,
                                 skip_group_check=True)
                # ---- s-step: TS = E * IH (2x bf16, 4 c-chunks); c+rr sums on
                # PE as a 32-matmul PSUM accumulation group
                # first chunk tiny so the PE pipeline starts early and the
                # post-DVE PE tail stays short
                pS = ps_s2.tile([B, NK], f32, tag="pS")
                mm_last = None
                cs = 0
                for ci, csz in enumerate((2, 10, 10, 10)):
                    TS = work.tile([128, csz, DC, NSH], bf16, tag="TS%d" % ci)
                    nc.vector.tensor_mul(
                        TS[:],
                        IH[:, cs:cs + csz],
                        E[:, cs:cs + csz, :].rearrange("p c (o n) -> p c o n", o=1)
                         .broadcast_to([128, csz, DC, NSH]),
                    )
                    for j in range(csz):
                        c = cs + j
                        mm_last = nc.tensor.matmul(
                            pS[:], bd4b_t[:],
                            TS[:, j].rearrange("p k n -> p (k n)"),
                            start=(c == 0), stop=(c == CH - 1),
                            skip_group_check=True,
                        )
                    cs += csz
                Zs = small.tile([B, NSH], f32, tag="Zs")
                nc.vector.tensor_copy(Zs[:], pz)
                Rz = small.tile([B, NSH], f32, tag="Rz")
                nc.vector.reciprocal(Rz[:], Zs[:])
                absorb("v", pS[:2, :2])          # DVE clock <- pS (PE)
                Su = work.tile([B, NK], f32, tag="Su")
                nc.vector.tensor_mul(
                    Su[:].rearrange("p (k n) -> p k n", n=NSH),
                    pS[:].rearrange("p (k n) -> p k n", n=NSH),
                    Rz[:].rearrange("p (o n) -> p o n", o=1)
                       .broadcast_to([B, DC, NSH]),
                )
                S = work.tile([B, NK], f32, tag="S")
                nc.vector.tensor_add(S[:], Su[:], brep_t)
                if it < 3:
                    OUT = squash(S, str(it), out_dtype=bf16, out_tag="outb")
                    OUTr = replicate(OUT, str(it))
                else:
                    OUT = squash(S, str(it))
                    # output DMA on the ACT engine's HWDGE queues: fresh
                    # queue, so its single wait slot carries just the RAW.
                    o_dma = nc.scalar.dma_start(out=out_d[:], in_=OUT[:])
                    # Pre-absorb every final sem into the SYNC engine so the
                    # Tile kernel-tail drain needs <=1 wait (codegen limit).
                    f_scr = small.tile([2, 4], f32, tag="fin")
                    f_act = nc.scalar.copy(f_scr[:, 0:2], OUT[:2, :2])
                    f_dve = nc.vector.tensor_copy(f_scr[:, 2:4], OUT[:2, :2])
                    for fin in ([cf_dma] + s_dmas +
                                [mm_last, f_act, f_dve, o_dma]):
                        fnop = nc.sync.nop()
                        add_dep_helper(fnop.ins, fin.ins, sync=True,
                                       reason="absorb final sem for tail drain")

    return nc


def _pack_inputs(inputs, W, B_param):
    """Host-side shard + relayout. Returns list of 8 in_maps."""
    import ml_dtypes

    bf = ml_dtypes.bfloat16
    inputs = np.ascontiguousarray(inputs, dtype=np.float32)
    W = np.ascontiguousarray(W, dtype=np.float32)
    B_param = np.ascontiguousarray(B_param, dtype=np.float32)

    Wp = np.zeros((CH, NCP, DC, DIN), dtype=np.float32)
    Wp[:, :NC] = W
    Bp = np.zeros((NCP, DC), dtype=np.float32)
    Bp[:NC] = B_param

    # xt[(c,dc), dd, (b,rr)] = x[b, 4c+rr, 128dc+dd]
    x4 = inputs.reshape(B, CH, 4, 2, 128)           # b, c, rr, dc, dd
    xt = x4.transpose(1, 3, 4, 0, 2).reshape(CH * 2, 128, 128).astype(bf)
    bd4 = np.zeros((128, B), dtype=np.float32)
    bd4[np.arange(128), np.arange(128) // 4] = 1.0
    bd4t = bd4.T

    in_maps = []
    for core in range(NCORES):
        sl = slice(core * NSH, (core + 1) * NSH)
        Wc = Wp[:, sl]                               # c, n, k, d
        w5 = Wc.reshape(CH, NSH, DC, 2, 128)         # c n k dc dd
        # columns in (k, n) order
        wtc = w5.transpose(0, 3, 4, 2, 1).reshape(CH * 2, 128, NK).astype(bf)
        cstfc = np.zeros((128, 544), dtype=np.float32)
        cstfc[:, 0:B] = bd4
        cstfc[0:B, B:B + 128] = bd4t
        # brep in (k, n) order
        cstfc[0:B, B + 128:B + 128 + NK] = np.broadcast_to(
            Bp[sl].T.reshape(1, NK), (B, NK))
        xwc = np.concatenate([xt, wtc], axis=2)      # [64, 128, 512] bf16
        # partition-major: [d, cd, 512] so DMA reads are contiguous per row
        xwc = np.ascontiguousarray(xwc.transpose(1, 0, 2))
        in_maps.append(dict(xw=xwc, cstf=cstfc))
    return in_maps


def _run(inputs, W, B_param, trace=False):
    from concourse.bass_utils import run_bass_kernel_spmd

    if "nc" not in _cache:
        _cache["nc"] = _build_nc()
    nc = _cache["nc"]
    in_maps = _pack_inputs(inputs, W, B_param)
    res = run_bass_kernel_spmd(nc, in_maps, core_ids=list(range(NCORES)),
                               trace=trace)
    # out cols are (k, n): reshape + transpose back to [B, n, k]
    outs = [r["out"].reshape(B, DC, NSH).transpose(0, 2, 1)
            for r in res.results]
    full = np.concatenate(outs, axis=1)[:, :NC, :]
    return np.ascontiguousarray(full.astype(np.float32)), res


def kernel(inputs, W, B_param):
    out, _ = _run(inputs, W, B_param, trace=False)
    return out
